# revision 1
# baseline (speedup 1.0000x reference)
"""Trainium2 Bass kernel for Bengio03HighwayBiLm.

Model: L=2 layers x 2 directions of [width-4 conv over sequence (H=512 -> 512)
+ ReLU + 2 highway sublayers (512 -> 1024 split into nonlin/gate)].

Sharding: data-parallel over batch across 8 cores (4 batches/core), weights
replicated. On device everything runs in channels-on-partitions layout
([ch, pos]); the host pre-transposes the input / weights and post-transposes
the output, so the device kernel needs no transposes at all. The conv is 4
accumulated matmuls over a column-shifted padded activation window. All
matmuls run in float16 (full PE rate + fast weight load); PSUM accumulates
in fp32. ScalarE applies bias+ReLU / bias+Sigmoid straight out of PSUM; VectorE
does the 3-op highway combine x' = r + g*(x - r).
"""

import sys

for _p in ("/opt/trn_rl_repo", "/root/.axon_site/_ro/trn_rl_repo"):
    if _p not in sys.path:
        sys.path.append(_p)

from contextlib import ExitStack

import numpy as np

import concourse.bass as bass
import concourse.tile as tile
from concourse import bacc, bass_utils, mybir

F32 = mybir.dt.float32
F16 = mybir.dt.float16
AF = mybir.ActivationFunctionType

B, S, H = 32, 512, 512
L, NHW, WIDTH = 2, 2, 3
NCORES = 8
BL = B // NCORES          # batches per core
SP = S + 2 * WIDTH        # padded sequence length
HC = H // 128             # channel chunks

_CACHE = {}


def _build():
    if "nc" in _CACHE:
        return _CACHE["nc"]

    nc = bacc.Bacc("TRN2", target_bir_lowering=False, debug=False,
                   num_devices=NCORES)

    x_t = nc.dram_tensor("x_t", [BL, H, SP], F16, kind="ExternalInput").ap()
    convw = nc.dram_tensor("convw", [L, 2, 4, 128, HC, 512], F16,
                           kind="ExternalInput").ap()
    hww = nc.dram_tensor("hww", [L, 2, NHW, 8, 128, HC, 128], F16,
                         kind="ExternalInput").ap()
    convb = nc.dram_tensor("convb", [L, 2, 128, 4], F32,
                           kind="ExternalInput").ap()
    hwb = nc.dram_tensor("hwb", [L, 2, 128, NHW * 8], F32,
                         kind="ExternalInput").ap()
    padt = nc.dram_tensor("padt", [L, 2, 128, HC, 3], F16,
                          kind="ExternalInput").ap()
    out_t = nc.dram_tensor("out_t", [L, 2, BL, H, S], F16,
                           kind="ExternalOutput").ap()

    with tile.TileContext(nc) as tc, ExitStack() as ctx:
        sb = ctx.enter_context(tc.tile_pool(name="sb", bufs=2))
        ps = ctx.enter_context(tc.tile_pool(name="ps", bufs=8, space="PSUM"))
        dr = ctx.enter_context(tc.tile_pool(name="dr", bufs=1, space="DRAM"))

        xmid = []
        for d in range(2):
            m = dr.tile([BL, HC, 128, SP], F16, name=f"xmid{d}",
                        tag=f"xmid{d}")
            xmid.append(m)

        for (li, d) in [(0, 0), (0, 1), (1, 0), (1, 1)]:
            def load_xin(b):
                tiles = []
                for hc in range(HC):
                    t = sb.tile([128, SP], F16, name=f"xin_{li}{d}{b}{hc}",
                                tag="xin", bufs=14)
                    if li == 0:
                        nc.sync.dma_start(t[:],
                                          x_t[b, hc * 128:(hc + 1) * 128, :])
                    else:
                        nc.sync.dma_start(t[:], xmid[d][b, hc])
                    tiles.append(t)
                return tiles

            # interleave first-batch inputs with tap-0 weights so the
            # first matmul's deps land earliest in the DMA queues
            xin0 = []
            wc = [[None] * HC for _ in range(4)]
            for hc in range(HC):
                t = sb.tile([128, SP], F16, name=f"xin_{li}{d}0{hc}",
                            tag="xin", bufs=14)
                if li == 0:
                    nc.sync.dma_start(t[:], x_t[0, hc * 128:(hc + 1) * 128, :])
                else:
                    nc.sync.dma_start(t[:], xmid[d][0, hc])
                xin0.append(t)
                w = sb.tile([128, 512], F16, name=f"wc_{li}{d}0{hc}",
                            tag="wc", bufs=20)
                nc.sync.dma_start(w[:], convw[li, d, 0, :, hc])
                wc[0][hc] = w
            for j in range(1, 4):
                for hc in range(HC):
                    w = sb.tile([128, 512], F16, name=f"wc_{li}{d}{j}{hc}",
                                tag="wc", bufs=20)
                    nc.sync.dma_start(w[:], convw[li, d, j, :, hc])
                    wc[j][hc] = w
            wh = []
            for jh in range(NHW):
                row = []
                for gc in range(8):
                    t = sb.tile([128, HC, 128], F16,
                                name=f"wh_{li}{d}{jh}{gc}", tag="wh", bufs=20)
                    nc.sync.dma_start(t[:], hww[li, d, jh, gc])
                    row.append(t)
                wh.append(row)
            cb = sb.tile([128, 4], F32, name=f"cb_{li}{d}", tag="cb", bufs=2)
            nc.gpsimd.dma_start(cb[:], convb[li, d])
            hb = sb.tile([128, NHW * 8], F32, name=f"hb_{li}{d}", tag="hb",
                         bufs=2)
            nc.gpsimd.dma_start(hb[:], hwb[li, d])
            if li == 0:
                # pre-write layer-1 pad columns into xmid (off critical path)
                padf = sb.tile([128, HC, 3], F16, name=f"padf_{d}",
                               tag="padf", bufs=2)
                nc.gpsimd.dma_start(padf[:], padt[1, 0])
                padb = sb.tile([128, HC, 3], F16, name=f"padb_{d}",
                               tag="padb", bufs=2)
                nc.gpsimd.dma_start(padb[:], padt[1, 1])
                for b in range(BL):
                    for hc in range(HC):
                        nc.gpsimd.dma_start(xmid[d][b, hc, :, 0:WIDTH],
                                            padf[:, hc])
                        nc.gpsimd.dma_start(xmid[d][b, hc, :, WIDTH + S:SP],
                                            padb[:, hc])

            off = 0 if d == 0 else WIDTH

            for b in range(BL):
                xin = xin0 if b == 0 else load_xin(b)

                # conv: out[oc, pos] = relu(b + sum_{j,hc} wT[j,hc,oc] @ x[hc, pos+j])
                xcur = []
                for oc in range(4):
                    pt = ps.tile([128, 512], F32, name=f"cps_{li}{d}{b}{oc}",
                                 tag="ps")
                    k = 0
                    for j in range(4):
                        for hc in range(HC):
                            nc.tensor.matmul(
                                pt[:],
                                wc[j][hc][:, oc * 128:(oc + 1) * 128],
                                xin[hc][:, off + j:off + j + S],
                                start=(k == 0), stop=(k == 15))
                            k += 1
                    h = sb.tile([128, 512], F16, name=f"hf_{li}{d}{b}{oc}",
                                tag="hf", bufs=8)
                    nc.scalar.activation(h[:], pt[:], AF.Relu,
                                         bias=cb[:, oc:oc + 1])
                    xcur.append(h)

                # highway sublayers
                for jh in range(NHW):
                    rt = [None] * 4
                    gt = [None] * 4
                    for gc in (0, 4, 1, 5, 2, 6, 3, 7):
                        pt = ps.tile([128, 512], F32,
                                     name=f"hps_{li}{d}{b}{jh}{gc}", tag="ps")
                        for hc in range(HC):
                            nc.tensor.matmul(
                                pt[:],
                                wh[jh][gc][:, hc, :],
                                xcur[hc][:],
                                start=(hc == 0), stop=(hc == HC - 1))
                        if gc < 4:
                            r = sb.tile([128, 512], F16,
                                        name=f"rt_{li}{d}{b}{jh}{gc}",
                                        tag="rt", bufs=6)
                            nc.scalar.activation(r[:], pt[:], AF.Relu,
                                                 bias=hb[:, jh * 8 + gc:jh * 8 + gc + 1])
                            rt[gc] = r
                        else:
                            g = sb.tile([128, 512], F16,
                                        name=f"gt_{li}{d}{b}{jh}{gc}",
                                        tag="gt", bufs=6)
                            nc.scalar.activation(g[:], pt[:], AF.Sigmoid,
                                                 bias=hb[:, jh * 8 + gc:jh * 8 + gc + 1])
                            gt[gc - 4] = g
                    xnew = []
                    for hc in range(HC):
                        xo = sb.tile([128, 512], F16,
                                     name=f"xo_{li}{d}{b}{jh}{hc}",
                                     tag=f"xo{jh}", bufs=8)
                        nc.vector.tensor_sub(xo[:], xcur[hc][:], rt[hc][:])
                        nc.vector.tensor_mul(xo[:], gt[hc][:], xo[:])
                        nc.vector.tensor_add(xo[:], xo[:], rt[hc][:])
                        xnew.append(xo)
                    xcur = xnew

                for hc in range(HC):
                    if li == 0:
                        nc.sync.dma_start(xmid[d][b, hc, :, WIDTH:WIDTH + S],
                                          xcur[hc][:])
                    nc.gpsimd.dma_start(
                        out_t[li, d, b, hc * 128:(hc + 1) * 128, :],
                        xcur[hc][:])

    nc.compile()
    _CACHE["nc"] = nc
    return nc


def _prep_shared(fwd_pad, bwd_pad, fwd_w, fwd_b, bwd_w, bwd_b,
                 fwd_hw_w, fwd_hw_b, bwd_hw_w, bwd_hw_b):
    f32 = np.float32
    convw = np.empty((L, 2, 4, 128, HC, 512), np.float16)
    convb = np.empty((L, 2, 128, 4), f32)
    hww = np.empty((L, 2, NHW, 8, 128, HC, 128), np.float16)
    hwb = np.empty((L, 2, 128, NHW * 8), f32)
    padt = np.empty((L, 2, 128, HC, 3), np.float16)
    for li in range(L):
        for d, (w, bia, hw_w, hw_b) in enumerate(
                [(fwd_w, fwd_b, fwd_hw_w, fwd_hw_b),
                 (bwd_w, bwd_b, bwd_hw_w, bwd_hw_b)]):
            # w[li]: [512o, 2048=(j,hc,p)] -> [j, p, hc, o]
            convw[li, d] = w[li].reshape(512, 4, HC, 128).transpose(1, 3, 2, 0)
            convb[li, d] = bia[li].reshape(4, 128).T
            for jh in range(NHW):
                # hw_w[li,jh]: [1024=(gc,gi), 512=(hc,p)] -> [gc, p, hc, gi]
                hww[li, d, jh] = hw_w[li, jh].reshape(8, 128, HC, 128) \
                                             .transpose(0, 3, 2, 1)
                hwb[li, d][:, jh * 8:(jh + 1) * 8] = hw_b[li, jh].reshape(8, 128).T
        padt[li, 0] = fwd_pad[li].T.reshape(HC, 128, 3).transpose(1, 0, 2)
        padt[li, 1] = bwd_pad[li].T.reshape(HC, 128, 3).transpose(1, 0, 2)
    return dict(convw=convw, convb=convb, hww=hww, hwb=hwb, padt=padt)


def kernel(inputs, fwd_pad, bwd_pad, fwd_w, fwd_b, bwd_w, bwd_b,
           fwd_hw_w, fwd_hw_b, bwd_hw_w, bwd_hw_b, _trace=False):
    nc = _build()
    shared = _prep_shared(
        np.asarray(fwd_pad), np.asarray(bwd_pad),
        np.asarray(fwd_w), np.asarray(fwd_b),
        np.asarray(bwd_w), np.asarray(bwd_b),
        np.asarray(fwd_hw_w), np.asarray(fwd_hw_b),
        np.asarray(bwd_hw_w), np.asarray(bwd_hw_b))
    x = np.asarray(inputs, dtype=np.float32)

    in_maps = []
    for c in range(NCORES):
        xs = x[c * BL:(c + 1) * BL].transpose(0, 2, 1)  # [BL, H, S]
        xc = np.empty((BL, H, SP), np.float16)
        xc[:, :, WIDTH:WIDTH + S] = xs
        xc[:, :, 0:WIDTH] = np.asarray(fwd_pad)[0].T[None]
        xc[:, :, WIDTH + S:SP] = np.asarray(bwd_pad)[0].T[None]
        in_maps.append({"x_t": xc, **shared})

    res = bass_utils.run_bass_kernel_spmd(
        nc, in_maps, core_ids=list(range(NCORES)), trace=_trace)

    out = np.empty((L, B, S, 2 * H), np.float32)
    for c in range(NCORES):
        o = res.results[c]["out_t"].astype(np.float32)  # [L, 2, BL, H, S]
        for li in range(L):
            out[li, c * BL:(c + 1) * BL, :, :H] = o[li, 0].transpose(0, 2, 1)
            out[li, c * BL:(c + 1) * BL, :, H:] = o[li, 1].transpose(0, 2, 1)
    if _trace:
        kernel.last_exec_time_ns = res.exec_time_ns
        kernel.last_trace = (res.instructions_and_trace[1]
                             if res.instructions_and_trace else None)
    return out

